# revision 19
# baseline (speedup 1.0000x reference)
"""Trainium2 Bass kernel for nn_ConstraintAwareBiasing.

Computes bias[b, n, i, j] = temp[n] * (relu(relu(hi[b,i] + hj[b,j]) @ W2 + b2) @ W3 + b3)[n]
with hi = x @ W1[:128] + b1, hj = x @ W1[128:], masked by `mask`.

Strategy (8 NeuronCores):
  - Shard the (b, i) query axis: core = b*4 + chunk, each core owns 128 i-rows
    against all 512 j for one batch element.
  - Host precomputes hi/hj (tiny [512,128] matmuls), folds head_temperatures
    into W3, adds b3*temp and applies the mask on the host.
  - On device, per query row i:
      s1: h1 = relu(hjT + hi_col)    DVE tensor_scalar, all operands bf16 so
                                     the 4x_2p perf mode engages
      W2: p1 = W2^T @ h1             PE matmul -> PSUM (pairs of i share a
                                     2-bank PSUM tile)
      s3: h2 = relu(p1 + b2)         ACT/DVE PSUM->SBUF pass (split to
                                     balance engine occupancy)
      W3: p2[32c+16f : +16] += W3f^T @ h2
                                     PE matmul, col-tiled 4x; two phases f
                                     with zero-padded M=32 stationaries
                                     ([W3|0] then accumulate [0|W3]) pack
                                     EIGHT i-rows into one PSUM bank
      s5: ot = copy(p2) bf16         one PSUM->SBUF pass per 8 rows
      DMA ships each ot slab (bf16, halves output bytes); host up-casts,
      reorders, and adds b3/mask.
    Stage 2 of group g-1 is emitted interleaved with stage 1 of group g
    (software pipelining) so the in-order engine streams don't block.
"""

import numpy as np
import ml_dtypes

import concourse.bass as bass
import concourse.tile as tile
import concourse.mybir as mybir
from concourse import bacc
from concourse.bass_utils import run_bass_kernel_spmd

BF16 = ml_dtypes.bfloat16

B, S, D = 2, 512, 128          # batch, seq, state dim
H, NH = 128, 16                # hidden, heads
N_CORES = 8
CHUNKS = N_CORES // B          # i-chunks per batch element
I_PER_CORE = S // CHUNKS       # 128
GROUPS = I_PER_CORE // 4       # 4 i-rows per group; 2 groups share a p2 bank
NEG_INF = float("-inf")

_CACHE: dict = {}

# Engine-assignment patterns (tuned against NTFF profiles).
# s1 per i (i % len): "v" = VectorE, "g" = GpSimdE
S1_PAT = ["v"] * 4
# s3 per triple index (t % len): "a" = ScalarE, "v" = VectorE.
# DVE also carries s1 + s5, so ACT takes ~9/10 of s3.
S3_PAT = ["a", "a", "a", "a", "v", "a", "a", "a", "a", "a"]
# s5 per 8-i slab (sl % len)
S5_PAT = ["v", "v"]


def _build_bass():
    nc = bacc.Bacc("TRN2")
    dt = mybir.dt
    hj_d = nc.dram_tensor("hj", (H, S), dt.bfloat16, kind="ExternalInput")
    hi_d = nc.dram_tensor("hi", (H, I_PER_CORE), dt.float32, kind="ExternalInput")
    w2_d = nc.dram_tensor("w2", (H, H), dt.bfloat16, kind="ExternalInput")
    # Two zero-padded W3 stationaries: w3a = [W3 | 0], w3b = [0 | W3], each
    # [H, 32]. Phase a writes rows 32c..32c+15 of the bank, phase b
    # accumulates rows 32c+16..32c+31 (its top 16 rows add zero).
    w3a_d = nc.dram_tensor("w3a", (H, 32), dt.bfloat16, kind="ExternalInput")
    w3b_d = nc.dram_tensor("w3b", (H, 32), dt.bfloat16, kind="ExternalInput")
    b2_d = nc.dram_tensor("b2", (H, 1), dt.float32, kind="ExternalInput")
    out_d = nc.dram_tensor("out", (GROUPS // 2, H, S), dt.bfloat16,
                           kind="ExternalOutput")

    relu = mybir.ActivationFunctionType.Relu
    add, amax = mybir.AluOpType.add, mybir.AluOpType.max

    with tile.TileContext(nc) as tc:
        with tc.tile_pool(name="singles", bufs=1) as singles, \
             tc.tile_pool(name="h1p", bufs=12) as h1p, \
             tc.tile_pool(name="h2p", bufs=10) as h2p, \
             tc.tile_pool(name="otp", bufs=4) as otp, \
             tc.tile_pool(name="ps1", bufs=2, space="PSUM") as ps1, \
             tc.tile_pool(name="ps2", bufs=2, space="PSUM") as ps2:
            hj = singles.tile([H, S], dt.bfloat16)
            hi = singles.tile([H, I_PER_CORE], dt.float32)
            w2 = singles.tile([H, H], dt.bfloat16)
            w3a = singles.tile([H, 32], dt.bfloat16)
            w3b = singles.tile([H, 32], dt.bfloat16)
            b2 = singles.tile([H, 1], dt.float32)
            # dummy relu first: pulls the ~2.7us ACT table load into the
            # input-DMA wait window instead of serializing at the first s3
            warm = singles.tile([128, 1], dt.float32)
            nc.vector.memset(warm[:], 0.0)
            nc.scalar.activation(out=warm[:], in_=warm[:], func=relu)
            # interleave input loads over both HWDGE queues; w2/hj first
            # (they gate the first W2 matmul)
            nc.sync.dma_start(out=hj[:], in_=hj_d[:])
            nc.scalar.dma_start(out=w2[:], in_=w2_d[:])
            nc.scalar.dma_start(out=hi[:], in_=hi_d[:])
            nc.sync.dma_start(out=b2[:], in_=b2_d[:])
            nc.scalar.dma_start(out=w3a[:], in_=w3a_d[:])
            nc.sync.dma_start(out=w3b[:], in_=w3b_d[:])
            # PE warm-up: ~4us of dummy matmuls on a memset tile during the
            # input-DMA wait flips the HAM clock gate to 8/8 before the first
            # real matmul (cold MMs run at 1.2 GHz instead of 2.4).
            wsrc = singles.tile([128, S], dt.bfloat16)
            nc.gpsimd.memset(wsrc[:], 0.0)
            wp = ps2.tile([128, S], dt.float32, name="wp", tag="p2")
            for _ in range(7):
                nc.tensor.matmul(wp[:], lhsT=wsrc[:, 0:128], rhs=wsrc[:],
                                 start=True, stop=True)

            # Software pipeline: stage1 (s1, W2, s3) walks i in TRIPLES
            # (FD=1536 evacuations amortize the ACT/DVE per-instruction
            # overhead); stage2 (W3 matmuls, s5, DMA) walks 4-i groups,
            # emitted with ~8 rows of slack behind the s3 frontier so the
            # 4 col-tiled W3 matmuls of a group never wait on h2.
            triples = [(i, min(i + 3, I_PER_CORE))
                       for i in range(0, I_PER_CORE, 3)]
            h2_of = {}              # i -> (h2_tile, col offset)
            state = {"next_g": 0, "done_i": 0, "p2": None}
            SLACK_I = 8

            def stage2(g):
                phase = g % 2
                if phase == 0:
                    state["p2"] = ps2.tile([128, S], dt.float32,
                                           name="p2", tag="p2")
                p2 = state["p2"]
                w3f = w3a if phase == 0 else w3b
                for c in range(4):
                    h2t, off = h2_of.pop(4 * g + c)
                    nc.tensor.matmul(
                        p2[32 * c:32 * c + 32, :], lhsT=w3f[:],
                        rhs=h2t[:, off:off + S],
                        start=(phase == 0), stop=(phase == 1),
                        tile_position=(0, 32 * c))
                if phase == 1:
                    sl = (g - 1) // 2
                    ot = otp.tile([128, S], dt.bfloat16, name="ot", tag="ot")
                    if S5_PAT[sl % len(S5_PAT)] == "v":
                        nc.vector.tensor_copy(ot[:], p2[:])
                    else:
                        nc.scalar.copy(out=ot[:], in_=p2[:])
                    nc.sync.dma_start(out=out_d[sl], in_=ot[:])

            for t, (i0, i1) in enumerate(triples):
                w = i1 - i0
                q = ps1.tile([H, 3 * S], dt.float32, name="q", tag="q")
                h2t = h2p.tile([H, 3 * S], dt.bfloat16, name="h2", tag="h2")
                for k in range(w):
                    i = i0 + k
                    h1 = h1p.tile([H, S], dt.bfloat16)
                    s1_eng = {"v": nc.vector,
                              "g": nc.gpsimd}[S1_PAT[i % len(S1_PAT)]]
                    s1_eng.tensor_scalar(
                        out=h1[:], in0=hj[:], scalar1=hi[:, i:i + 1],
                        scalar2=0.0, op0=add, op1=amax)
                    nc.tensor.matmul(
                        q[:, k * S:(k + 1) * S],
                        lhsT=w2[:], rhs=h1[:], start=True, stop=True)
                    h2_of[i] = (h2t, k * S)
                if S3_PAT[t % len(S3_PAT)] == "a":
                    nc.scalar.activation(out=h2t[:, :w * S], in_=q[:, :w * S],
                                         func=relu, bias=b2[:], scale=1.0)
                else:
                    nc.vector.tensor_scalar(
                        out=h2t[:, :w * S], in0=q[:, :w * S],
                        scalar1=b2[:, 0:1], scalar2=0.0, op0=add, op1=amax)
                state["done_i"] = i1
                while (state["next_g"] < GROUPS
                       and 4 * state["next_g"] + 4 + SLACK_I <= state["done_i"]):
                    stage2(state["next_g"])
                    state["next_g"] += 1
            while state["next_g"] < GROUPS:
                stage2(state["next_g"])
                state["next_g"] += 1
    nc.compile()
    return nc


def _host_prep(inputs):
    x = np.asarray(inputs["state_embeddings"], dtype=np.float32)   # [B, S, D]
    W1 = np.asarray(inputs["W1"], dtype=np.float32)                # [2D, H]
    b1 = np.asarray(inputs["b1"], dtype=np.float32)                # [H]
    W2 = np.asarray(inputs["W2"], dtype=np.float32)                # [H, H]
    b2 = np.asarray(inputs["b2"], dtype=np.float32)                # [H]
    W3 = np.asarray(inputs["W3"], dtype=np.float32)                # [H, NH]
    b3 = np.asarray(inputs["b3"], dtype=np.float32)                # [NH]
    temp = np.asarray(inputs["head_temperatures"], dtype=np.float32)  # [NH]

    hi = x @ W1[:D] + b1                                           # [B, S, H]
    hj = x @ W1[D:]                                                # [B, S, H]
    w3p = (W3 * temp[None, :]).astype(BF16)                        # temp folded in
    b3p = b3 * temp                                                # added on host

    w3pad = np.zeros((H, 32), dtype=BF16)
    w3pad[:, :NH] = w3p
    w3a = np.ascontiguousarray(w3pad)
    w3b = np.ascontiguousarray(np.roll(w3pad, NH, axis=1))

    b2col = np.ascontiguousarray(b2.reshape(H, 1))

    in_maps = []
    for core in range(N_CORES):
        b, chunk = divmod(core, CHUNKS)
        i0 = chunk * I_PER_CORE
        in_maps.append({
            "hj": np.ascontiguousarray(hj[b].T).astype(BF16),                  # [H, S]
            "hi": np.ascontiguousarray(hi[b, i0:i0 + I_PER_CORE].T,
                                       dtype=np.float32),                      # [H, I]
            "w2": W2.astype(BF16),
            "w3a": w3a,
            "w3b": w3b,
            "b2": b2col,
        })
    return in_maps, b3p


def _assemble(results, inputs, b3p):
    mask = np.asarray(inputs["mask"])
    out = np.empty((B, NH, S, S), dtype=np.float32)
    for core in range(N_CORES):
        b, chunk = divmod(core, CHUNKS)
        i0 = chunk * I_PER_CORE
        # core result: [slab, 128, S] bf16; partition 32c+16f+n holds
        # (i = 8*slab + 4f + c, head n)
        r = results[core]["out"].astype(np.float32)
        r = r.reshape(GROUPS // 2, 4, 2, NH, S)        # [sl, c, f, n, j]
        r = r.transpose(3, 0, 2, 1, 4)                 # [n, sl, f, c, j]
        out[b, :, i0:i0 + I_PER_CORE, :] = r.reshape(NH, I_PER_CORE, S)
    if b3p.any():
        out += b3p[None, :, None, None]
    if not mask.all():
        out = np.where(mask[:, None, :, :], out, np.float32(NEG_INF))
    return out


def _get_nc():
    if "nc" not in _CACHE:
        _CACHE["nc"] = _build_bass()
    return _CACHE["nc"]


def run(inputs, trace=False):
    nc = _get_nc()
    in_maps, b3p = _host_prep(inputs)
    res = run_bass_kernel_spmd(nc, in_maps, core_ids=list(range(N_CORES)),
                               trace=trace)
    out = _assemble(res.results, inputs, b3p)
    return out, res


def kernel(**inputs) -> np.ndarray:
    out, _ = run(inputs, trace=False)
    return out
